# revision 8
# baseline (speedup 1.0000x reference)
"""FRAUDRE InterAgg (3-relation GNN message passing) on 8 TRN2 NeuronCores.

v8: bulk SWDGE gather/scatter with 4-way queue-parallel descriptor generation.

Key facts (measured on HW):
  - Extended SWDGE instructions (DMAGatherAnt / DMAScatterAddAnt) dispatch in
    ~70ns and run their descriptor generation on the Q7 cpu pair owning their
    queue (cpu_id/2 == queue_num). Instructions on different queues overlap
    fully (4x). Mainline indirect DMA (the v4 baseline) blocks the engine.
  - Desc-gen costs ~8ns/token (gather) and ~7ns/token (scatter) per pair.
  - dma_scatter_add RMW races for duplicate destinations (lost updates), so
    scatter destinations are unique cells, written exactly once into
    pre-zeroed window accumulators.

Structure per core (1024 batch nodes, 98304 neighbor tokens):
  - bf16 pair table [500k, 128] (host cast; row pv = nodes 2pv, 2pv+1),
    16 segments of 32768 rows for int16 gather indices.
  - 48 buckets (segment x relation). Per bucket: one dma_gather (packed,
    trailing -1 skipped via per-core GPR count), a Vector parity select, and
    one dma_scatter_add placing each selected row at unique cell j*1024+n in
    the relation's 32768-cell window (parity-split acc pair per window).
  - Buckets pipeline across the 4 SWDGE queues (bucket k on queue k%4) with
    6-deep gather lookahead so all 4 Q7 pairs stay busy.
  - Vector tree-sums each group's 32 member cells (stride-4 columns), then
    the tail: out = [self | relu(self) | sum_r w_r/32 * relu(sum_r)].
  - Self features ride the baseline's exact-f32 indirect-DMA path.
"""

import numpy as np


def _import_concourse():
    try:
        import concourse.bass  # noqa: F401
    except ImportError:
        import sys

        for p in ("/opt/trn_rl_repo", "/root/.axon_site/_ro/trn_rl_repo"):
            if p not in sys.path:
                sys.path.insert(0, p)
        import concourse.bass  # noqa: F401


N_CORES = 8
NUM_NODES = 1_000_000
EMBED = 64
N_BATCH = 8192
DEG = 32
PER_CORE = N_BATCH // N_CORES  # 1024
P = 128
NQ = 4

NPAIR = NUM_NODES // 2
SEG_ROWS = 32768
NSEG = 16
LAST_ROWS = NPAIR - 15 * SEG_ROWS  # 8480
N_TILES = PER_CORE // P  # 8
NBUCKET = 3 * NSEG  # 48
LOOKAHEAD = 8
SEL_RING = 6


def build_nc(caps):
    """caps: list[48] of per-bucket capacities (multiples of 128). Bucket
    k = 3*s + r covers (segment s, relation r)."""
    _import_concourse()
    from contextlib import ExitStack

    import concourse.bacc as bacc
    import concourse.bass as bass
    import concourse.mybir as mybir

    f32 = mybir.dt.float32
    bf16 = mybir.dt.bfloat16
    i16 = mybir.dt.int16
    i32 = mybir.dt.int32
    Exp = mybir.ActivationFunctionType.Exp

    blocks = [c // P for c in caps]
    icols = [c // 16 for c in caps]
    IDX_TOT = sum(icols)
    MASK_TOT = sum(blocks)
    BMAX = max(blocks)

    # scatters of different buckets write disjoint cells of shared window
    # accumulators; the race detector can't see that, so it is off.
    nc = bacc.Bacc(
        "TRN2",
        num_swdge_queues=NQ,
        dynamic_dma_scratch_size=16384,
        detect_race_conditions=False,
    )
    table = nc.dram_tensor("table", [NPAIR, 2 * EMBED], bf16, kind="ExternalInput")
    alpha = nc.dram_tensor("alpha", [2 * EMBED, 3], f32, kind="ExternalInput")
    feats32 = nc.dram_tensor("feats32", [NUM_NODES, EMBED], f32, kind="ExternalInput")
    nodes = nc.dram_tensor("nodes", [PER_CORE], i32, kind="ExternalInput")
    gidx_d = nc.dram_tensor("gidx", [P, IDX_TOT], i16, kind="ExternalInput")
    sidx_d = nc.dram_tensor("sidx", [P, IDX_TOT], i16, kind="ExternalInput")
    mask_d = nc.dram_tensor("mask", [P, MASK_TOT], bf16, kind="ExternalInput")
    cnts_d = nc.dram_tensor("cnts", [1, NBUCKET], i32, kind="ExternalInput")
    out = nc.dram_tensor("out", [PER_CORE, 3 * EMBED], f32, kind="ExternalOutput")
    w_dram = nc.dram_tensor("w_scratch", [2 * EMBED, 3], f32)

    ioff = [0] * (NBUCKET + 1)
    boff = [0] * (NBUCKET + 1)
    for k in range(NBUCKET):
        ioff[k + 1] = ioff[k] + icols[k]
        boff[k + 1] = boff[k] + blocks[k]

    def seg_rows(s):
        return SEG_ROWS if s < NSEG - 1 else LAST_ROWS

    with ExitStack() as ctx:
        e = ctx.enter_context

        alpha_sb = e(nc.sbuf_tensor([2 * EMBED, 3], f32))
        w_e = e(nc.sbuf_tensor([2 * EMBED, 3], f32))
        w_s = e(nc.sbuf_tensor([2 * EMBED, 1], f32))
        w_rs = e(nc.sbuf_tensor([2 * EMBED, 1], f32))
        w_sb = e(nc.sbuf_tensor([2 * EMBED, 3], f32))
        wb_sb = e(nc.sbuf_tensor([P, 3 * EMBED], f32))

        nodes_all = e(nc.sbuf_tensor([P, N_TILES], i32))
        gidx_sb = e(nc.sbuf_tensor([P, IDX_TOT], i16))
        sidx_sb = e(nc.sbuf_tensor([P, IDX_TOT], i16))
        mask_sb = e(nc.sbuf_tensor([P, MASK_TOT], bf16))
        cnts_sb = e(nc.sbuf_tensor([1, NBUCKET], i32))

        gbuf = [
            e(nc.sbuf_tensor(f"gbuf{i}", [P, BMAX * 2 * EMBED], bf16))
            for i in range(LOOKAHEAD)
        ]
        selb = [
            e(nc.sbuf_tensor(f"selb{i}", [P, BMAX * EMBED], bf16))
            for i in range(SEL_RING)
        ]
        tmpb = e(nc.sbuf_tensor([P, BMAX * EMBED], bf16))
        acc_e = [e(nc.sbuf_tensor(f"acc_e{r}", [P, 128 * EMBED], bf16)) for r in range(3)]
        acc_o = [e(nc.sbuf_tensor(f"acc_o{r}", [P, 128 * EMBED], bf16)) for r in range(3)]
        work = e(nc.sbuf_tensor([P, 16 * EMBED], f32))
        s_rq = e(nc.sbuf_tensor([P, EMBED], f32))
        tmp64 = e(nc.sbuf_tensor([P, EMBED], f32))
        out_sb = [
            e(nc.sbuf_tensor(f"out_sb{t}", [P, 3 * EMBED], f32)) for t in range(N_TILES)
        ]

        in_sem = e(nc.semaphore("in_sem"))
        alpha_sem = e(nc.semaphore("alpha_sem"))
        nodes_sem = e(nc.semaphore("nodes_sem"))
        e_sem = e(nc.semaphore("e_sem"))
        v_sem = e(nc.semaphore("v_sem"))
        wd_sem = e(nc.semaphore("wd_sem"))
        wb_sem = e(nc.semaphore("wb_sem"))
        z_sem = e(nc.semaphore("z_sem"))
        g_sems = [e(nc.semaphore(f"g_sem{q}")) for q in range(NQ)]
        sel_sem = e(nc.semaphore("sel_sem"))
        sc_sems = [e(nc.semaphore(f"sc_sem{q}")) for q in range(NQ)]
        gs_sems = [e(nc.semaphore(f"gs_sem{q}")) for q in range(NQ)]
        red_sem = e(nc.semaphore("red_sem"))
        st_sem = e(nc.semaphore("st_sem"))

        block = e(nc.Block())

        def nq_count(k_hi, q):
            return len([k for k in range(k_hi + 1) if k % NQ == q])

        @block.sync
        def _(sync):
            sync.dma_start(out=alpha_sb[:], in_=alpha[:, :]).then_inc(alpha_sem, 16)
            with nc.allow_non_contiguous_dma(reason="one-time 4KB index load"):
                sync.dma_start(
                    out=nodes_all[:], in_=nodes[:].rearrange("(t p) -> p t", p=P)
                ).then_inc(nodes_sem, 16)
            sync.dma_start(out=gidx_sb[:], in_=gidx_d[:, :]).then_inc(in_sem, 16)
            sync.dma_start(out=sidx_sb[:], in_=sidx_d[:, :]).then_inc(in_sem, 16)
            sync.dma_start(out=mask_sb[:], in_=mask_d[:, :]).then_inc(in_sem, 16)
            sync.dma_start(out=cnts_sb[:], in_=cnts_d[:, :]).then_inc(in_sem, 16)
            sync.wait_ge(v_sem, 1)
            sync.dma_start(out=w_dram[:, :], in_=w_sb[:]).then_inc(wd_sem, 16)
            for t in range(N_TILES):
                sync.wait_ge(red_sem, t + 1)
                sync.dma_start(
                    out=out[t * P : (t + 1) * P, :], in_=out_sb[t][:]
                ).then_inc(st_sem, 16)

        @block.scalar
        def _(scalar):
            scalar.wait_ge(alpha_sem, 16)
            scalar.activation(w_e[:], alpha_sb[:], Exp).then_inc(e_sem, 1)
            for r in range(3):
                scalar.memzero(acc_e[r][:])
                scalar.memzero(acc_o[r][:])
            scalar.add(w_e[0:1, 0:1], w_e[0:1, 0:1], 0.0).then_inc(z_sem, 1)

        @block.gpsimd
        def _(gpsimd):
            # self gathers first (mainline SWDGE: blocking dispatch, ~1.1us ea)
            gpsimd.wait_ge(nodes_sem, 16)
            for t in range(N_TILES):
                inst = gpsimd.indirect_dma_start(
                    out=out_sb[t][:, 0:EMBED],
                    out_offset=None,
                    in_=feats32[:],
                    in_offset=bass.IndirectOffsetOnAxis(
                        ap=nodes_all[:, t : t + 1], axis=0
                    ),
                )
                inst.ins.queue = f"qPoolDynamic{t % NQ or ''}"
                inst.ins.single_packet = True
                inst.then_inc(gs_sems[t % NQ], 16)

            gpsimd.wait_ge(in_sem, 64)
            cnt = nc.gpsimd.alloc_register("cnt")

            def gather(k):
                s = k // 3
                gpsimd.load(cnt, cnts_sb[0:1, k : k + 1])
                gpsimd.dma_gather(
                    out_ap=gbuf[k % LOOKAHEAD][:]
                    .rearrange("p (b e) -> p b e", e=2 * EMBED)[:, : blocks[k], :],
                    in_ap=table[s * SEG_ROWS : s * SEG_ROWS + seg_rows(s), :],
                    idxs_ap=gidx_sb[:, ioff[k] : ioff[k + 1]],
                    num_idxs=caps[k],
                    num_idxs_reg=cnt,
                    elem_size=2 * EMBED,
                    single_packet=False,
                    queue_num=k % NQ,
                ).then_inc(g_sems[k % NQ], 16)

            def scatter(k):
                r = k % 3
                gpsimd.load(cnt, cnts_sb[0:1, k : k + 1])
                gpsimd.dma_scatter_add(
                    out_ap=acc_e[r][:].rearrange("p (g e) -> p g e", e=EMBED),
                    in_ap=selb[k % SEL_RING][:]
                    .rearrange("p (b e) -> p b e", e=EMBED)[:, : blocks[k], :],
                    idxs_ap=sidx_sb[:, ioff[k] : ioff[k + 1]],
                    num_idxs=caps[k],
                    num_idxs_reg=cnt,
                    elem_size=EMBED,
                    sbuf_tokens_per_rank=128,
                    parity_reg=0,
                    out_ap_other=acc_o[r][:].rearrange("p (g e) -> p g e", e=EMBED),
                    single_packet=False,
                    queue_num=k % NQ,
                ).then_inc(sc_sems[k % NQ], 16)

            # warmup: two full rounds of gathers
            for k in range(2 * NQ):
                gather(k)
            gpsimd.wait_ge(z_sem, 1)
            # steady state: rounds of [4 scatters][4 gathers]
            nrounds = NBUCKET // NQ
            for rnd in range(nrounds):
                for q in range(NQ):
                    k = rnd * NQ + q
                    gpsimd.wait_ge(sel_sem, k + 1)
                    scatter(k)
                for q in range(NQ):
                    k = (rnd + 2) * NQ + q
                    if k < NBUCKET:
                        # gbuf ring safety: select (k - LOOKAHEAD) released it
                        if k >= LOOKAHEAD:
                            gpsimd.wait_ge(sel_sem, k - LOOKAHEAD + 1)
                        gather(k)

            # weight broadcast (needed only by the reduction tail)
            gpsimd.wait_ge(wd_sem, 16)
            gpsimd.dma_start(
                out=wb_sb[:],
                in_=w_dram[EMBED : 2 * EMBED, :]
                .rearrange("f r -> (f r)")[None, :]
                .partition_broadcast(P),
            ).then_inc(wb_sem, 16)

        @block.vector
        def _(vector):
            vector.wait_ge(e_sem, 1)
            vector.reduce_sum(w_s[:], w_e[:], axis=mybir.AxisListType.X)
            vector.drain()
            vector.reciprocal(w_rs[:], w_s[:])
            vector.drain()
            vector.tensor_mul(w_sb[:], w_e[:], w_rs[:].to_broadcast([2 * EMBED, 3]))
            vector.drain()
            vector.tensor_scalar_mul(w_sb[:], w_sb[:], 1.0 / DEG).then_inc(v_sem, 1)


            # parity selects
            for k in range(NBUCKET):
                vector.wait_ge(g_sems[k % NQ], 16 * (k // NQ + 1))
                if k >= SEL_RING:
                    # sel buffer reuse: scatter k-SEL_RING must have read it
                    kp = k - SEL_RING
                    vector.wait_ge(sc_sems[kp % NQ], 16 * (kp // NQ + 1))
                g3 = gbuf[k % LOOKAHEAD][:].rearrange("p (b e) -> p b e", e=2 * EMBED)[
                    :, : blocks[k], :
                ]
                s3 = selb[k % SEL_RING][:].rearrange("p (b e) -> p b e", e=EMBED)[
                    :, : blocks[k], :
                ]
                t3 = tmpb[:].rearrange("p (b e) -> p b e", e=EMBED)[:, : blocks[k], :]
                m3 = (
                    mask_sb[:, boff[k] : boff[k + 1]]
                    .rearrange("p b -> p b ()")
                    .to_broadcast([P, blocks[k], EMBED])
                )
                vector.tensor_sub(t3, g3[:, :, EMBED : 2 * EMBED], g3[:, :, 0:EMBED])
                vector.drain()
                vector.tensor_mul(t3, t3, m3)
                vector.drain()
                vector.tensor_add(s3, t3, g3[:, :, 0:EMBED]).then_inc(sel_sem, 1)

            for q in range(NQ):
                vector.wait_ge(sc_sems[q], 16 * nq_count(NBUCKET - 1, q))
                n_g = len([t for t in range(N_TILES) if t % NQ == q])
                if n_g:
                    vector.wait_ge(gs_sems[q], 16 * n_g)
            vector.wait_ge(wb_sem, 16)

            for t in range(N_TILES):
                ob = out_sb[t]
                vector.tensor_relu(ob[:, EMBED : 2 * EMBED], ob[:, 0:EMBED])
                acc_out = ob[:, 2 * EMBED : 3 * EMBED]
                for r in range(3):
                    src = acc_e[r] if t % 2 == 0 else acc_o[r]
                    base = t // 2
                    a3 = src[:].rearrange("p (c e) -> p c e", e=EMBED)
                    w3 = work[:].rearrange("p (c e) -> p c e", e=EMBED)
                    vector.tensor_add(
                        w3, a3[:, base:128:8, :], a3[:, base + 4 : 128 : 8, :]
                    )
                    vector.drain()
                    width = 16
                    while width > 2:
                        half = width // 2
                        vector.tensor_add(
                            w3[:, 0:half, :], w3[:, 0:half, :], w3[:, half:width, :]
                        )
                        vector.drain()
                        width = half
                    vector.tensor_add(s_rq[:], w3[:, 0, :], w3[:, 1, :])
                    vector.drain()
                    vector.tensor_relu(s_rq[:], s_rq[:])
                    vector.drain()
                    wb_r = wb_sb[:, r : 3 * EMBED : 3]
                    if r == 0:
                        vector.tensor_mul(acc_out, s_rq[:], wb_r)
                    else:
                        vector.tensor_mul(tmp64[:], s_rq[:], wb_r)
                        vector.drain()
                        vector.tensor_add(acc_out, acc_out, tmp64[:])
                    vector.drain()
                vector.engine_nop().then_inc(red_sem, 1)

    nc.compile()
    return nc


def _wrap16(x, n):
    w = np.zeros((P, n // 16), dtype=np.int16)
    b = x.reshape(n // 16, 16).T
    for rep in range(8):
        w[rep * 16 : (rep + 1) * 16, :] = b
    return w


def _plan_core(nis, caps):
    """Gather/scatter metadata for one core. nis: [3][1024, 32] neighbor ids."""
    import ml_dtypes

    gidx_parts, sidx_parts, mask_parts = [], [], []
    cnts = np.zeros(NBUCKET, dtype=np.int32)
    v = np.stack([np.asarray(nis[r]) for r in range(3)])
    pv = (v >> 1).astype(np.int64)
    par = (v & 1).astype(np.int64)
    seg = pv // SEG_ROWS
    inseg = pv - seg * SEG_ROWS
    n_idx = np.broadcast_to(np.arange(1024)[None, :, None], v.shape)
    j_idx = np.broadcast_to(np.arange(DEG)[None, None, :], v.shape)
    cell = j_idx * 1024 + n_idx

    for k in range(NBUCKET):
        s, r = k // 3, k % 3
        cap = caps[k]
        m = seg[r] == s
        c = int(m.sum())
        assert 0 < c <= cap, (k, c, cap)
        cnts[k] = c
        gi = np.full(cap, -1, dtype=np.int16)
        gi[:c] = inseg[r][m].astype(np.int16)
        si = np.full(cap, -1, dtype=np.int16)
        si[:c] = cell[r][m].astype(np.int16)
        mk = np.zeros(cap, dtype=ml_dtypes.bfloat16)
        mk[:c] = par[r][m].astype(np.float32)
        gidx_parts.append(_wrap16(gi, cap))
        sidx_parts.append(_wrap16(si, cap))
        mw = np.zeros((P, cap // P), dtype=ml_dtypes.bfloat16)
        idx = np.arange(cap)
        mw[idx % P, idx // P] = mk
        mask_parts.append(mw)

    return {
        "gidx": np.concatenate(gidx_parts, axis=1),
        "sidx": np.concatenate(sidx_parts, axis=1),
        "mask": np.concatenate(mask_parts, axis=1),
        "cnts": cnts.reshape(1, NBUCKET),
    }


_NC_CACHE = {}


def _run(inputs, trace=False, trace_kwargs=None):
    _import_concourse()
    import ml_dtypes
    from concourse.bass_utils import run_bass_kernel_spmd

    features = np.ascontiguousarray(np.asarray(inputs["features"], dtype=np.float32))
    alpha = np.ascontiguousarray(np.asarray(inputs["alpha"], dtype=np.float32))
    nodes = np.asarray(inputs["nodes"]).astype(np.int32)
    nis = [np.asarray(inputs[f"neigh_idx{r + 1}"]).astype(np.int64) for r in range(3)]

    table = features.astype(ml_dtypes.bfloat16).reshape(NPAIR, 2 * EMBED)

    counts = np.zeros((NBUCKET,), dtype=np.int64)
    for r in range(3):
        seg = (nis[r] >> 1) // SEG_ROWS
        for c in range(N_CORES):
            sl = seg[c * PER_CORE : (c + 1) * PER_CORE]
            for s in range(NSEG):
                counts[3 * s + r] = max(counts[3 * s + r], (sl == s).sum())
    caps = [max(int(-(-int(c) // P) * P), P) for c in counts]
    key = tuple(caps)

    if key not in _NC_CACHE:
        _NC_CACHE[key] = build_nc(list(caps))
    nc = _NC_CACHE[key]

    in_maps = []
    for c in range(N_CORES):
        sl = slice(c * PER_CORE, (c + 1) * PER_CORE)
        plan = _plan_core([nis[r][sl] for r in range(3)], caps)
        in_maps.append(
            {
                "table": table,
                "alpha": alpha,
                "feats32": features,
                "nodes": np.ascontiguousarray(nodes[sl]),
                **plan,
            }
        )

    kw = {}
    if trace:
        kw["trace"] = True
        if trace_kwargs:
            kw.update(trace_kwargs)
    res = run_bass_kernel_spmd(nc, in_maps, list(range(N_CORES)), **kw)
    out_full = np.concatenate([res.results[c]["out"] for c in range(N_CORES)], axis=0)
    return out_full, res


def kernel(**inputs) -> np.ndarray:
    out, _ = _run(inputs)
    return out


# revision 9
# speedup vs baseline: 1.1251x; 1.1251x over previous
"""FRAUDRE InterAgg (3-relation GNN message passing) on 8 TRN2 NeuronCores.

v8: bulk SWDGE gather/scatter with 4-way queue-parallel descriptor generation.

Key facts (measured on HW):
  - Extended SWDGE instructions (DMAGatherAnt / DMAScatterAddAnt) dispatch in
    ~70ns and run their descriptor generation on the Q7 cpu pair owning their
    queue (cpu_id/2 == queue_num). Instructions on different queues overlap
    fully (4x). Mainline indirect DMA (the v4 baseline) blocks the engine.
  - Desc-gen costs ~8ns/token (gather) and ~7ns/token (scatter) per pair.
  - dma_scatter_add RMW races for duplicate destinations (lost updates), so
    scatter destinations are unique cells, written exactly once into
    pre-zeroed window accumulators.

Structure per core (1024 batch nodes, 98304 neighbor tokens):
  - bf16 pair table [500k, 128] (host cast; row pv = nodes 2pv, 2pv+1),
    16 segments of 32768 rows for int16 gather indices.
  - 48 buckets (segment x relation). Per bucket: one dma_gather (packed,
    trailing -1 skipped via per-core GPR count), a Vector parity select, and
    one dma_scatter_add placing each selected row at unique cell j*1024+n in
    the relation's 32768-cell window (parity-split acc pair per window).
  - Buckets pipeline across the 4 SWDGE queues (bucket k on queue k%4) with
    6-deep gather lookahead so all 4 Q7 pairs stay busy.
  - Vector tree-sums each group's 32 member cells (stride-4 columns), then
    the tail: out = [self | relu(self) | sum_r w_r/32 * relu(sum_r)].
  - Self features ride the baseline's exact-f32 indirect-DMA path.
"""

import numpy as np


def _import_concourse():
    try:
        import concourse.bass  # noqa: F401
    except ImportError:
        import sys

        for p in ("/opt/trn_rl_repo", "/root/.axon_site/_ro/trn_rl_repo"):
            if p not in sys.path:
                sys.path.insert(0, p)
        import concourse.bass  # noqa: F401


N_CORES = 8
NUM_NODES = 1_000_000
EMBED = 64
N_BATCH = 8192
DEG = 32
PER_CORE = N_BATCH // N_CORES  # 1024
P = 128
NQ = 4

NPAIR = NUM_NODES // 2
SEG_ROWS = 32768
NSEG = 16
LAST_ROWS = NPAIR - 15 * SEG_ROWS  # 8480
N_TILES = PER_CORE // P  # 8
NBUCKET = 3 * NSEG  # 48
LOOKAHEAD = 8
SEL_RING = 8


def build_nc(caps):
    """caps: list[48] of per-bucket capacities (multiples of 128). Bucket
    k = 3*s + r covers (segment s, relation r)."""
    _import_concourse()
    from contextlib import ExitStack

    import concourse.bacc as bacc
    import concourse.bass as bass
    import concourse.mybir as mybir

    f32 = mybir.dt.float32
    bf16 = mybir.dt.bfloat16
    i16 = mybir.dt.int16
    i32 = mybir.dt.int32
    Exp = mybir.ActivationFunctionType.Exp

    blocks = [c // P for c in caps]
    icols = [c // 16 for c in caps]
    IDX_TOT = sum(icols)
    MASK_TOT = sum(blocks)
    BMAX = max(blocks)

    # scatters of different buckets write disjoint cells of shared window
    # accumulators; the race detector can't see that, so it is off.
    nc = bacc.Bacc(
        "TRN2",
        num_swdge_queues=NQ,
        dynamic_dma_scratch_size=16384,
        detect_race_conditions=False,
    )
    table = nc.dram_tensor("table", [NPAIR, 2 * EMBED], bf16, kind="ExternalInput")
    alpha = nc.dram_tensor("alpha", [2 * EMBED, 3], f32, kind="ExternalInput")
    feats32 = nc.dram_tensor("feats32", [NUM_NODES, EMBED], f32, kind="ExternalInput")
    nodes = nc.dram_tensor("nodes", [PER_CORE], i32, kind="ExternalInput")
    gidx_d = nc.dram_tensor("gidx", [P, IDX_TOT], i16, kind="ExternalInput")
    sidx_d = nc.dram_tensor("sidx", [P, IDX_TOT], i16, kind="ExternalInput")
    mask_d = nc.dram_tensor("mask", [P, MASK_TOT], bf16, kind="ExternalInput")
    cnts_d = nc.dram_tensor("cnts", [1, NBUCKET], i32, kind="ExternalInput")
    out = nc.dram_tensor("out", [PER_CORE, 3 * EMBED], f32, kind="ExternalOutput")
    w_dram = nc.dram_tensor("w_scratch", [2 * EMBED, 3], f32)

    ioff = [0] * (NBUCKET + 1)
    boff = [0] * (NBUCKET + 1)
    for k in range(NBUCKET):
        ioff[k + 1] = ioff[k] + icols[k]
        boff[k + 1] = boff[k] + blocks[k]

    def seg_rows(s):
        return SEG_ROWS if s < NSEG - 1 else LAST_ROWS

    with ExitStack() as ctx:
        e = ctx.enter_context

        alpha_sb = e(nc.sbuf_tensor([2 * EMBED, 3], f32))
        w_e = e(nc.sbuf_tensor([2 * EMBED, 3], f32))
        w_s = e(nc.sbuf_tensor([2 * EMBED, 1], f32))
        w_rs = e(nc.sbuf_tensor([2 * EMBED, 1], f32))
        w_sb = e(nc.sbuf_tensor([2 * EMBED, 3], f32))
        wb_sb = e(nc.sbuf_tensor([P, 3 * EMBED], f32))

        nodes_all = e(nc.sbuf_tensor([P, N_TILES], i32))
        gidx_sb = e(nc.sbuf_tensor([P, IDX_TOT], i16))
        sidx_sb = e(nc.sbuf_tensor([P, IDX_TOT], i16))
        mask_sb = e(nc.sbuf_tensor([P, MASK_TOT], bf16))
        cnts_sb = e(nc.sbuf_tensor([1, NBUCKET], i32))

        gbuf = [
            e(nc.sbuf_tensor(f"gbuf{i}", [P, BMAX * 2 * EMBED], bf16))
            for i in range(LOOKAHEAD)
        ]
        selb = [
            e(nc.sbuf_tensor(f"selb{i}", [P, BMAX * EMBED], bf16))
            for i in range(SEL_RING)
        ]
        tmpb = e(nc.sbuf_tensor([P, BMAX * EMBED], bf16))
        acc_e = [e(nc.sbuf_tensor(f"acc_e{r}", [P, 128 * EMBED], bf16)) for r in range(3)]
        acc_o = [e(nc.sbuf_tensor(f"acc_o{r}", [P, 128 * EMBED], bf16)) for r in range(3)]
        work = e(nc.sbuf_tensor([P, 16 * EMBED], f32))
        s_rq = e(nc.sbuf_tensor([P, EMBED], f32))
        tmp64 = e(nc.sbuf_tensor([P, EMBED], f32))
        out_sb = [
            e(nc.sbuf_tensor(f"out_sb{t}", [P, 3 * EMBED], f32)) for t in range(N_TILES)
        ]

        in_sem = e(nc.semaphore("in_sem"))
        alpha_sem = e(nc.semaphore("alpha_sem"))
        nodes_sem = e(nc.semaphore("nodes_sem"))
        e_sem = e(nc.semaphore("e_sem"))
        v_sem = e(nc.semaphore("v_sem"))
        wd_sem = e(nc.semaphore("wd_sem"))
        wb_sem = e(nc.semaphore("wb_sem"))
        z_sem = e(nc.semaphore("z_sem"))
        g_sems = [e(nc.semaphore(f"g_sem{q}")) for q in range(NQ)]
        sel_sem = e(nc.semaphore("sel_sem"))
        sc_sems = [e(nc.semaphore(f"sc_sem{q}")) for q in range(NQ)]
        gs_sems = [e(nc.semaphore(f"gs_sem{q}")) for q in range(NQ)]
        red_sem = e(nc.semaphore("red_sem"))
        st_sem = e(nc.semaphore("st_sem"))

        block = e(nc.Block())

        def nq_count(k_hi, q):
            return len([k for k in range(k_hi + 1) if k % NQ == q])

        @block.sync
        def _(sync):
            sync.dma_start(out=alpha_sb[:], in_=alpha[:, :]).then_inc(alpha_sem, 16)
            with nc.allow_non_contiguous_dma(reason="one-time 4KB index load"):
                sync.dma_start(
                    out=nodes_all[:], in_=nodes[:].rearrange("(t p) -> p t", p=P)
                ).then_inc(nodes_sem, 16)
            sync.dma_start(out=gidx_sb[:], in_=gidx_d[:, :]).then_inc(in_sem, 16)
            sync.dma_start(out=sidx_sb[:], in_=sidx_d[:, :]).then_inc(in_sem, 16)
            sync.dma_start(out=mask_sb[:], in_=mask_d[:, :]).then_inc(in_sem, 16)
            sync.dma_start(out=cnts_sb[:], in_=cnts_d[:, :]).then_inc(in_sem, 16)
            sync.wait_ge(v_sem, 1)
            sync.dma_start(out=w_dram[:, :], in_=w_sb[:]).then_inc(wd_sem, 16)
            for t in range(N_TILES):
                sync.wait_ge(red_sem, t + 1)
                sync.dma_start(
                    out=out[t * P : (t + 1) * P, :], in_=out_sb[t][:]
                ).then_inc(st_sem, 16)

        @block.scalar
        def _(scalar):
            scalar.wait_ge(alpha_sem, 16)
            scalar.activation(w_e[:], alpha_sb[:], Exp).then_inc(e_sem, 1)
            for r in range(3):
                scalar.memzero(acc_e[r][:])
                scalar.memzero(acc_o[r][:])
            scalar.add(w_e[0:1, 0:1], w_e[0:1, 0:1], 0.0).then_inc(z_sem, 1)

        @block.gpsimd
        def _(gpsimd):
            # self gathers first (mainline SWDGE: blocking dispatch, ~1.1us ea)
            gpsimd.wait_ge(nodes_sem, 16)
            for t in range(N_TILES):
                inst = gpsimd.indirect_dma_start(
                    out=out_sb[t][:, 0:EMBED],
                    out_offset=None,
                    in_=feats32[:],
                    in_offset=bass.IndirectOffsetOnAxis(
                        ap=nodes_all[:, t : t + 1], axis=0
                    ),
                )
                inst.ins.queue = f"qPoolDynamic{t % NQ or ''}"
                inst.ins.single_packet = True
                inst.then_inc(gs_sems[t % NQ], 16)

            gpsimd.wait_ge(in_sem, 64)
            cnt = nc.gpsimd.alloc_register("cnt")

            def gather(k):
                s = k // 3
                gpsimd.load(cnt, cnts_sb[0:1, k : k + 1])
                gpsimd.dma_gather(
                    out_ap=gbuf[k % LOOKAHEAD][:]
                    .rearrange("p (b e) -> p b e", e=2 * EMBED)[:, : blocks[k], :],
                    in_ap=table[s * SEG_ROWS : s * SEG_ROWS + seg_rows(s), :],
                    idxs_ap=gidx_sb[:, ioff[k] : ioff[k + 1]],
                    num_idxs=caps[k],
                    num_idxs_reg=cnt,
                    elem_size=2 * EMBED,
                    single_packet=False,
                    queue_num=k % NQ,
                ).then_inc(g_sems[k % NQ], 16)

            def scatter(k):
                r = k % 3
                gpsimd.load(cnt, cnts_sb[0:1, k : k + 1])
                gpsimd.dma_scatter_add(
                    out_ap=acc_e[r][:].rearrange("p (g e) -> p g e", e=EMBED),
                    in_ap=selb[k % SEL_RING][:]
                    .rearrange("p (b e) -> p b e", e=EMBED)[:, : blocks[k], :],
                    idxs_ap=sidx_sb[:, ioff[k] : ioff[k + 1]],
                    num_idxs=caps[k],
                    num_idxs_reg=cnt,
                    elem_size=EMBED,
                    sbuf_tokens_per_rank=128,
                    parity_reg=0,
                    out_ap_other=acc_o[r][:].rearrange("p (g e) -> p g e", e=EMBED),
                    single_packet=False,
                    queue_num=k % NQ,
                ).then_inc(sc_sems[k % NQ], 16)

            for k in range(LOOKAHEAD):
                gather(k)
            gpsimd.wait_ge(z_sem, 1)
            for k in range(NBUCKET):
                gpsimd.wait_ge(sel_sem, k + 1)
                scatter(k)
                if k + LOOKAHEAD < NBUCKET:
                    gather(k + LOOKAHEAD)

            # weight broadcast (needed only by the reduction tail)
            gpsimd.wait_ge(wd_sem, 16)
            gpsimd.dma_start(
                out=wb_sb[:],
                in_=w_dram[EMBED : 2 * EMBED, :]
                .rearrange("f r -> (f r)")[None, :]
                .partition_broadcast(P),
            ).then_inc(wb_sem, 16)

        @block.vector
        def _(vector):
            vector.wait_ge(e_sem, 1)
            vector.reduce_sum(w_s[:], w_e[:], axis=mybir.AxisListType.X)
            vector.drain()
            vector.reciprocal(w_rs[:], w_s[:])
            vector.drain()
            vector.tensor_mul(w_sb[:], w_e[:], w_rs[:].to_broadcast([2 * EMBED, 3]))
            vector.drain()
            vector.tensor_scalar_mul(w_sb[:], w_sb[:], 1.0 / DEG).then_inc(v_sem, 1)


            # parity selects
            for k in range(NBUCKET):
                vector.wait_ge(g_sems[k % NQ], 16 * (k // NQ + 1))
                if k >= SEL_RING:
                    # sel buffer reuse: scatter k-SEL_RING must have read it
                    kp = k - SEL_RING
                    vector.wait_ge(sc_sems[kp % NQ], 16 * (kp // NQ + 1))
                g3 = gbuf[k % LOOKAHEAD][:].rearrange("p (b e) -> p b e", e=2 * EMBED)[
                    :, : blocks[k], :
                ]
                s3 = selb[k % SEL_RING][:].rearrange("p (b e) -> p b e", e=EMBED)[
                    :, : blocks[k], :
                ]
                t3 = tmpb[:].rearrange("p (b e) -> p b e", e=EMBED)[:, : blocks[k], :]
                m3 = (
                    mask_sb[:, boff[k] : boff[k + 1]]
                    .rearrange("p b -> p b ()")
                    .to_broadcast([P, blocks[k], EMBED])
                )
                vector.tensor_sub(t3, g3[:, :, EMBED : 2 * EMBED], g3[:, :, 0:EMBED])
                vector.drain()
                vector.tensor_mul(t3, t3, m3)
                vector.drain()
                vector.tensor_add(s3, t3, g3[:, :, 0:EMBED]).then_inc(sel_sem, 1)

            for q in range(NQ):
                vector.wait_ge(sc_sems[q], 16 * nq_count(NBUCKET - 1, q))
                n_g = len([t for t in range(N_TILES) if t % NQ == q])
                if n_g:
                    vector.wait_ge(gs_sems[q], 16 * n_g)
            vector.wait_ge(wb_sem, 16)

            for t in range(N_TILES):
                ob = out_sb[t]
                vector.tensor_relu(ob[:, EMBED : 2 * EMBED], ob[:, 0:EMBED])
                acc_out = ob[:, 2 * EMBED : 3 * EMBED]
                for r in range(3):
                    src = acc_e[r] if t % 2 == 0 else acc_o[r]
                    base = t // 2
                    a3 = src[:].rearrange("p (c e) -> p c e", e=EMBED)
                    w3 = work[:].rearrange("p (c e) -> p c e", e=EMBED)
                    vector.tensor_add(
                        w3, a3[:, base:128:8, :], a3[:, base + 4 : 128 : 8, :]
                    )
                    vector.drain()
                    width = 16
                    while width > 2:
                        half = width // 2
                        vector.tensor_add(
                            w3[:, 0:half, :], w3[:, 0:half, :], w3[:, half:width, :]
                        )
                        vector.drain()
                        width = half
                    vector.tensor_add(s_rq[:], w3[:, 0, :], w3[:, 1, :])
                    vector.drain()
                    vector.tensor_relu(s_rq[:], s_rq[:])
                    vector.drain()
                    wb_r = wb_sb[:, r : 3 * EMBED : 3]
                    if r == 0:
                        vector.tensor_mul(acc_out, s_rq[:], wb_r)
                    else:
                        vector.tensor_mul(tmp64[:], s_rq[:], wb_r)
                        vector.drain()
                        vector.tensor_add(acc_out, acc_out, tmp64[:])
                    vector.drain()
                vector.engine_nop().then_inc(red_sem, 1)

    nc.compile()
    return nc


def _wrap16(x, n):
    w = np.zeros((P, n // 16), dtype=np.int16)
    b = x.reshape(n // 16, 16).T
    for rep in range(8):
        w[rep * 16 : (rep + 1) * 16, :] = b
    return w


def _plan_core(nis, caps):
    """Gather/scatter metadata for one core. nis: [3][1024, 32] neighbor ids."""
    import ml_dtypes

    gidx_parts, sidx_parts, mask_parts = [], [], []
    cnts = np.zeros(NBUCKET, dtype=np.int32)
    v = np.stack([np.asarray(nis[r]) for r in range(3)])
    pv = (v >> 1).astype(np.int64)
    par = (v & 1).astype(np.int64)
    seg = pv // SEG_ROWS
    inseg = pv - seg * SEG_ROWS
    n_idx = np.broadcast_to(np.arange(1024)[None, :, None], v.shape)
    j_idx = np.broadcast_to(np.arange(DEG)[None, None, :], v.shape)
    cell = j_idx * 1024 + n_idx

    for k in range(NBUCKET):
        s, r = k // 3, k % 3
        cap = caps[k]
        m = seg[r] == s
        c = int(m.sum())
        assert 0 < c <= cap, (k, c, cap)
        cnts[k] = c
        gi = np.full(cap, -1, dtype=np.int16)
        gi[:c] = inseg[r][m].astype(np.int16)
        si = np.full(cap, -1, dtype=np.int16)
        si[:c] = cell[r][m].astype(np.int16)
        mk = np.zeros(cap, dtype=ml_dtypes.bfloat16)
        mk[:c] = par[r][m].astype(np.float32)
        gidx_parts.append(_wrap16(gi, cap))
        sidx_parts.append(_wrap16(si, cap))
        mw = np.zeros((P, cap // P), dtype=ml_dtypes.bfloat16)
        idx = np.arange(cap)
        mw[idx % P, idx // P] = mk
        mask_parts.append(mw)

    return {
        "gidx": np.concatenate(gidx_parts, axis=1),
        "sidx": np.concatenate(sidx_parts, axis=1),
        "mask": np.concatenate(mask_parts, axis=1),
        "cnts": cnts.reshape(1, NBUCKET),
    }


_NC_CACHE = {}


def _run(inputs, trace=False, trace_kwargs=None):
    _import_concourse()
    import ml_dtypes
    from concourse.bass_utils import run_bass_kernel_spmd

    features = np.ascontiguousarray(np.asarray(inputs["features"], dtype=np.float32))
    alpha = np.ascontiguousarray(np.asarray(inputs["alpha"], dtype=np.float32))
    nodes = np.asarray(inputs["nodes"]).astype(np.int32)
    nis = [np.asarray(inputs[f"neigh_idx{r + 1}"]).astype(np.int64) for r in range(3)]

    table = features.astype(ml_dtypes.bfloat16).reshape(NPAIR, 2 * EMBED)

    counts = np.zeros((NBUCKET,), dtype=np.int64)
    for r in range(3):
        seg = (nis[r] >> 1) // SEG_ROWS
        for c in range(N_CORES):
            sl = seg[c * PER_CORE : (c + 1) * PER_CORE]
            for s in range(NSEG):
                counts[3 * s + r] = max(counts[3 * s + r], (sl == s).sum())
    caps = [max(int(-(-int(c) // P) * P), P) for c in counts]
    key = tuple(caps)

    if key not in _NC_CACHE:
        _NC_CACHE[key] = build_nc(list(caps))
    nc = _NC_CACHE[key]

    in_maps = []
    for c in range(N_CORES):
        sl = slice(c * PER_CORE, (c + 1) * PER_CORE)
        plan = _plan_core([nis[r][sl] for r in range(3)], caps)
        in_maps.append(
            {
                "table": table,
                "alpha": alpha,
                "feats32": features,
                "nodes": np.ascontiguousarray(nodes[sl]),
                **plan,
            }
        )

    kw = {}
    if trace:
        kw["trace"] = True
        if trace_kwargs:
            kw.update(trace_kwargs)
    res = run_bass_kernel_spmd(nc, in_maps, list(range(N_CORES)), **kw)
    out_full = np.concatenate([res.results[c]["out"] for c in range(N_CORES)], axis=0)
    return out_full, res


def kernel(**inputs) -> np.ndarray:
    out, _ = _run(inputs)
    return out
